# revision 27
# baseline (speedup 1.0000x reference)
"""Trainium2 Bass kernel for nn_BiLSTMDecoderModel.

Strategy (8 NeuronCores, data-parallel over batch, B=128 -> 16 rows/core):
  * backward LSTM: only b_hs[0] is consumed downstream == ONE cell step on x_0.
  * forward LSTM: 256-step scan. Recurrent matmul uses h-stationary layout
    (LDWEIGHTS cost ~ M=16 cols) with 4-way PE column tiling so the four
    512-col gate blocks stream concurrently through 4 XBUSes.
  * all gate nonlinearities collapse to tanh via sigmoid(x)=(tanh(x/2)+1)/2;
    the 1/2 factors are folded into host-preprocessed weights. The carried
    cell state is z=2c and hidden is h'=2h (Whh pre-scaled by 0.5).
  * per-step gate math runs in hidden-on-partitions layout (after 4 PE
    transposes), which directly yields the transposed-h stationary for the
    next step's matmuls.
  * input projections (phase A) are computed INTERLEAVED with the scan, one
    chunk per step, so the PE never idles long enough for the HAM clock gate
    to re-throttle it to 1.2 GHz, and phase A's cost is absorbed into the
    scan's engine-idle windows. xproj stays in SBUF (8 steps per [128,2048]
    tile); per-step gate injection reads 32-row pair slices through a
    selector stationary (even/odd row picker) so all PE row offsets stay
    32-aligned.
  * gates accumulate in two PSUM half-tiles (hidden 0:256 / 256:512) so the
    tanh of half A overlaps the matmul waves of half B.
  * input-weight biases ride along as an extra ones-row in the embedding
    stationary (K=45 third chunk) instead of separate bias matmuls.
  * decoder: gi (input-gate projections) computed once during the scan tail,
    scattered to 32-aligned partitions by DMA, injected per class as K=1
    matmuls; the n-gate bias is applied with one broadcast STT + one tanh.
"""

import sys

sys.path.insert(0, "/opt/trn_rl_repo")

import numpy as np
import ml_dtypes

import concourse.bass as bass
import concourse.mybir as mybir
import concourse.tile as tile
from concourse import bacc
from concourse.bass_utils import run_bass_kernel_spmd
from concourse.masks import make_identity

V, E, H, NCLS = 100000, 300, 512, 6
B, S = 128, 256
NC = 8
BL = B // NC  # 16
G4 = 4 * H  # 2048
G3 = 3 * 2 * H  # 3072
H2 = 2 * H  # 1024

f32 = mybir.dt.float32
bf16 = mybir.dt.bfloat16
i32 = mybir.dt.int32
Tanh = mybir.ActivationFunctionType.Tanh
Exp = mybir.ActivationFunctionType.Exp
Ln = mybir.ActivationFunctionType.Ln
Ident = mybir.ActivationFunctionType.Identity
ADD = mybir.AluOpType.add
SUB = mybir.AluOpType.subtract
MUL = mybir.AluOpType.mult
MAX = mybir.AluOpType.max

_cache = {}


def _bf(x):
    return np.ascontiguousarray(x.astype(ml_dtypes.bfloat16))


def _build_program():
    nc = bacc.Bacc(
        "TRN2", target_bir_lowering=False, debug=False, enable_asserts=False,
        num_devices=NC,
    )
    # ---- DRAM I/O ----
    embedW_d = nc.dram_tensor("embedW", [V, E], f32, kind="ExternalInput").ap()
    idx_d = nc.dram_tensor("idx", [128, 32], i32, kind="ExternalInput").ap()
    wihT_d = nc.dram_tensor("wihT", [304, G4], bf16, kind="ExternalInput").ap()
    bwihT_d = nc.dram_tensor("bwihT", [304, G4], bf16, kind="ExternalInput").ap()
    whhT_d = nc.dram_tensor("whhT", [H, G4], bf16, kind="ExternalInput").ap()
    dwhhT_d = nc.dram_tensor("dwhhT", [1028, G3], bf16, kind="ExternalInput").ap()
    dwihT_d = nc.dram_tensor("dwihT", [516, G3], bf16, kind="ExternalInput").ap()
    ecw_d = nc.dram_tensor("ecw", [NCLS, H], f32, kind="ExternalInput").ap()
    clsT_d = nc.dram_tensor("clsT", [1028, 2], bf16, kind="ExternalInput").ap()
    sel_d = nc.dram_tensor("sel", [128, 32], bf16, kind="ExternalInput").ap()
    out_d = nc.dram_tensor("out", [NCLS, BL, 2], f32, kind="ExternalOutput").ap()

    with tile.TileContext(nc) as tc:
        _emit(nc, tc, embedW_d, idx_d, wihT_d, bwihT_d, whhT_d, dwhhT_d,
              dwihT_d, ecw_d, clsT_d, sel_d, out_d)
    nc.compile()
    return nc


def _emit(nc, tc, embedW_d, idx_d, wihT_d, bwihT_d, whhT_d, dwhhT_d, dwihT_d,
          ecw_d, clsT_d, sel_d, out_d):
    def pool(**kw):
        return tc.alloc_tile_pool(**kw)

    const = pool(name="const", bufs=1)

    # ---- persistent SBUF constants ----
    ident = const.tile([128, 128], f32, tag="ident", name="ident")
    make_identity(nc, ident[:])
    identb = const.tile([128, 128], bf16, tag="identb", name="identb")
    make_identity(nc, identb[:])
    sel_sb = const.tile([128, 32], bf16, tag="sel", name="sel")
    nc.sync.dma_start(sel_sb[:], sel_d[:])
    ones128 = const.tile([128, 16], bf16, tag="ones128", name="ones128")
    nc.gpsimd.memset(ones128[:], 1.0)
    bias_stat = const.tile([4, 16], bf16, tag="bias_stat", name="bias_stat")
    nc.gpsimd.memset(bias_stat[:], 0.0)
    nc.gpsimd.memset(bias_stat[0:1, :], 1.0)

    def tr(out_ap, in_ap, pin):
        nc.tensor.transpose(out_ap, in_ap, ident[0:pin, 0:pin])

    idx_sb = const.tile([128, 32], i32, tag="idx", name="idx")
    nc.sync.dma_start(idx_sb[:], idx_d[:])

    # input weights; chunk2 holds rows 256:301 = 44 weight rows + the bias
    # row at local row 44 (bias folded into K via a ones row in embT_b).
    wih_sb = [const.tile([128, G4], bf16, tag=f"wih{k}", name=f"wih{k}") for k in range(3)]
    bwih_sb = [const.tile([128, G4], bf16, tag=f"bwih{k}", name=f"bwih{k}") for k in range(3)]
    for k in range(2):
        nc.sync.dma_start(wih_sb[k][:], wihT_d[128 * k:128 * (k + 1), :])
        nc.sync.dma_start(bwih_sb[k][:], bwihT_d[128 * k:128 * (k + 1), :])
    nc.sync.dma_start(wih_sb[2][0:45, :], wihT_d[256:301, :])
    nc.sync.dma_start(bwih_sb[2][0:45, :], bwihT_d[256:301, :])

    whh_sb = [const.tile([128, G4], bf16, tag=f"whh{k}", name=f"whh{k}") for k in range(4)]
    for k in range(4):
        nc.sync.dma_start(whh_sb[k][:], whhT_d[128 * k:128 * (k + 1), :])

    dwhh_sb = [const.tile([128, G3], bf16, tag=f"dwhh{k}", name=f"dwhh{k}") for k in range(9)]
    for k in range(8):
        nc.sync.dma_start(dwhh_sb[k][:], dwhhT_d[128 * k:128 * (k + 1), :])
    nc.sync.dma_start(dwhh_sb[8][0:4, :], dwhhT_d[1024:1028, :])

    dwih_sb = [const.tile([128, G3], bf16, tag=f"dwih{k}", name=f"dwih{k}") for k in range(5)]
    for k in range(4):
        nc.sync.dma_start(dwih_sb[k][:], dwihT_d[128 * k:128 * (k + 1), :])
    nc.sync.dma_start(dwih_sb[4][0:4, :], dwihT_d[512:516, :])

    cls_sb = [const.tile([128, 2], bf16, tag=f"cls{k}", name=f"cls{k}") for k in range(9)]
    for k in range(8):
        nc.sync.dma_start(cls_sb[k][:], clsT_d[128 * k:128 * (k + 1), :])
    nc.sync.dma_start(cls_sb[8][0:4, :], clsT_d[1024:1028, :])

    ce_t = const.tile([NCLS, H], f32, tag="ce", name="ce")
    nc.sync.dma_start(ce_t[:], ecw_d[:])

    # state tiles that persist across phases
    bH = const.tile([128, 64], bf16, tag="bH", name="bH")  # backward-cell h' (2h)

    # decoder-prep SBUF (lives until the decoder; allocated early so the
    # scan-phase pools above it can be released in LIFO order)
    pDP = pool(name="pDP", bufs=1)
    ce2 = pDP.tile([NCLS, H], f32, tag="ce2", name="ce2")
    ceT = pDP.tile([128, 24], bf16, tag="ceT", name="ceT")
    gi_sb = pDP.tile([NCLS, G3], bf16, tag="gi", name="gi")
    gi_sc = [pDP.tile([128, G3], bf16, tag=f"gisc{i}", name=f"gisc{i}")
             for i in range(2)]
    giT = pDP.tile([128, 48], f32, tag="giT", name="giT")
    Hd0 = pDP.tile([128, 128], bf16, tag="Hd0", name="Hd0")

    # ======== Phase A machinery (interleaved into the scan) ========
    pA = pool(name="pA", bufs=3)       # gather / tanh / embT tiles
    pXA = pool(name="pXA", bufs=4)     # xproj SBUF tiles, one per 8 steps
    pPS = pool(name="pPS", bufs=2, space="PSUM")  # shared A/decoder-prep PSUM

    xa_tiles = {}

    def emit_gather(m):
        g_t = pA.tile([128, 304], f32, tag="gath", name="gath")
        nc.gpsimd.indirect_dma_start(
            out=g_t[:, 0:E],
            out_offset=None,
            in_=embedW_d[:],
            in_offset=bass.IndirectOffsetOnAxis(ap=idx_sb[:, m:m + 1], axis=0),
        )
        return g_t

    def emit_tanh_emb(m, g_t):
        th = pA.tile([128, 304], f32, tag="th", name="th")
        nc.scalar.activation(th[:, 0:E], g_t[:, 0:E], Tanh)
        # ones column -> becomes the bias ones-row after the transpose
        nc.vector.memset(th[:, 300:301], 1.0)
        return th

    def emit_embT(m, th):
        pst = pPS.tile([128, 512], f32, tag="pps", name="pst")
        tr(pst[0:128, 0:128], th[:, 0:128], 128)
        tr(pst[0:128, 128:256], th[:, 128:256], 128)
        tr(pst[0:45, 256:384], th[:, 256:301], 128)
        embT_a = pA.tile([128, 256], bf16, tag="embTa", name="embTa")
        nc.vector.tensor_copy(embT_a[:], pst[:, 0:256])
        embT_b = pA.tile([48, 128], bf16, tag="embTb", name="embTb")
        nc.vector.tensor_copy(embT_b[0:45, :], pst[0:45, 256:384])
        return embT_a, embT_b

    def emit_xmm(m, embT_a, embT_b, xa, psx_d, f, copy_eng="act"):
        b, kc = divmod(f, 3)
        nsl = slice(512 * b, 512 * (b + 1))
        if kc == 0:
            psx_d[b] = pPS.tile([128, 512], f32, tag="pps", name="psx")
        psx = psx_d[b]
        if kc == 0:
            nc.tensor.matmul(psx[:], embT_a[:, 0:128], wih_sb[0][:, nsl],
                             start=True, stop=False)
        elif kc == 1:
            nc.tensor.matmul(psx[:], embT_a[:, 128:256], wih_sb[1][:, nsl],
                             start=False, stop=False)
        else:
            nc.tensor.matmul(psx[:], embT_b[0:45, :], wih_sb[2][0:45, nsl],
                             start=False, stop=True)
            if copy_eng == "act":
                nc.scalar.activation(xa[:, nsl], psx[:], Ident)
            else:
                nc.vector.tensor_copy(xa[:, nsl], psx[:])

    # ---- prologue: phase A iters 0 and 1; backward cell on x_0 ----
    emb0 = None
    for m in (0, 1):
        g_t = emit_gather(m)
        th = emit_tanh_emb(m, g_t)
        ea, eb = emit_embT(m, th)
        xa = pXA.tile([128, G4], bf16, tag="xa", name="xa")
        xa_tiles[m] = xa
        psx_d = {}
        for f in range(12):
            emit_xmm(m, ea, eb, xa, psx_d, f)
        if m == 0:
            emb0 = (ea, eb)
    g2 = emit_gather(2)

    # backward LSTM single cell on x_0 (h=c=0)
    pB = pool(name="pB", bufs=1)
    pBps = pool(name="pBps", bufs=1, space="PSUM")
    ea0, eb0 = emb0
    bps = pBps.tile([128, 512], f32, tag="bps", name="bps")
    for j in range(4):
        ns = slice(512 * j, 512 * (j + 1))
        o = bps[32 * j:32 * j + 16, :]
        tp = (0, 32 * j)
        nc.tensor.matmul(o, ea0[:, 0:16], bwih_sb[0][:, ns],
                         start=True, stop=False, tile_position=tp)
        nc.tensor.matmul(o, ea0[:, 128:144], bwih_sb[1][:, ns],
                         start=False, stop=False, tile_position=tp)
        nc.tensor.matmul(o, eb0[0:45, 0:16], bwih_sb[2][0:45, ns],
                         start=False, stop=True, tile_position=tp)
    bT = pB.tile([128, 512], f32, tag="bT", name="bT")
    nc.scalar.activation(bT[0:112, :], bps[0:112, :], Tanh)
    bpt = pBps.tile([128, 448], f32, tag="bpt", name="bpt")
    for k in range(4):
        tr(bpt[:, 112 * k:112 * (k + 1)], bT[0:112, 128 * k:128 * (k + 1)], 112)
    bv = bpt[:].rearrange("p (c w) -> p c w", w=112)
    btip = pB.tile([128, 64], f32, tag="btip", name="btip")
    nc.scalar.activation(btip[:].rearrange("p (c w) -> p c w", w=16),
                         bv[:, :, 0:16], Ident, bias=1.0)
    bzv = pB.tile([128, 64], f32, tag="bzv", name="bzv")
    nc.vector.tensor_tensor(
        out=bzv[:].rearrange("p (c w) -> p c w", w=16),
        in0=btip[:].rearrange("p (c w) -> p c w", w=16),
        in1=bv[:, :, 64:80], op=MUL)
    btc = pB.tile([128, 64], f32, tag="btc", name="btc")
    nc.scalar.activation(btc[:], bzv[:], Tanh, scale=0.5)
    nc.vector.scalar_tensor_tensor(
        out=bH[:].rearrange("p (c w) -> p c w", w=16),
        in0=bv[:, :, 96:112], scalar=1.0,
        in1=btc[:].rearrange("p (c w) -> p c w", w=16),
        op0=ADD, op1=MUL)
    pBps.release()
    pB.release()

    # ======== Phase C: forward scan, 256 steps, phase A interleaved ========
    pH = pool(name="pH", bufs=2)
    pG = pool(name="pG", bufs=2, space="PSUM")   # psgA/psgB halves
    pT = pool(name="pT", bufs=2)
    pTT = pool(name="pTT", bufs=1, space="PSUM")  # transposed gates
    pHt = pool(name="pHt", bufs=1, space="PSUM")  # HAM-heater scratch
    pZ = pool(name="pZ", bufs=2)
    pW = pool(name="pW", bufs=2)

    # HAM heater: the PE clock-gate un-throttles (1.2 -> 2.4 GHz) only after
    # ~3.4us of SUSTAINED matmul activity and re-throttles after idle windows.
    # The scan's dependency chain leaves ~1.5us PE-idle per step, which keeps
    # the PE permanently throttled.  Dummy matmuls into a scratch PSUM bank
    # fill those windows so the whole scan runs at 2.4 GHz.
    heat = pHt.tile([128, 512], f32, tag="heat", name="heat")

    # Warm-up burst: ~70 dummy matmuls (≈6us of dense PE work) pinned after
    # the backward cell via a bH read, so they run right before the scan and
    # un-throttle the HAM clock gate; the scan itself then keeps the PE warm.
    nc.tensor.matmul(heat[0:16, 0:64], identb[:, 0:16], bH[:],
                     start=True, stop=True)
    for _ in range(70):
        nc.tensor.matmul(heat[0:16, 0:128], identb[:, 0:16],
                         whh_sb[0][:, 0:128], start=True, stop=True)

    z_prev = pZ.tile([128, 64], f32, tag="z", name="z")
    H_prev = pH.tile([128, 64], bf16, tag="H", name="H")
    nc.vector.memset(z_prev[:], 0.0)
    nc.vector.memset(H_prev[:], 0.0)

    def emit_dprep(k):
        if k == 0:
            nc.scalar.activation(ce2[:], ce_t[:], Tanh)
            psc = pPS.tile([128, 512], f32, tag="pps", name="psc")
            for kk in range(4):
                tr(psc[:, 6 * kk:6 * (kk + 1)],
                   ce2[0:NCLS, 128 * kk:128 * (kk + 1)], NCLS)
            nc.vector.tensor_copy(ceT[:], psc[:, 0:24])
        elif 1 <= k <= 6:
            ng = k - 1
            ns = slice(512 * ng, 512 * (ng + 1))
            psgi = pPS.tile([128, 512], f32, tag="pps", name="psgi")
            for kc in range(4):
                nc.tensor.matmul(psgi[0:NCLS, :], ceT[:, 6 * kc:6 * (kc + 1)],
                                 dwih_sb[kc][:, ns], start=(kc == 0), stop=False)
            nc.tensor.matmul(psgi[0:NCLS, :], bias_stat[0:4, 0:NCLS],
                             dwih_sb[4][0:4, ns], start=False, stop=True)
            nc.scalar.activation(gi_sb[:, ns], psgi[0:NCLS, :], Ident)
        elif k == 7:
            # transposed gi_n (n-gate bias in hidden-on-partitions layout)
            psgT = pPS.tile([128, 512], bf16, tag="pps", name="psgT")
            for gc in range(8):
                nc.tensor.transpose(
                    psgT[:, 6 * gc:6 * (gc + 1)],
                    gi_sb[0:NCLS, 2048 + 128 * gc:2048 + 128 * (gc + 1)],
                    identb[0:NCLS, 0:NCLS])
            nc.vector.tensor_copy(giT[:], psgT[:, 0:48])
        elif k == 8:
            # scatter gi rows to 32-aligned partitions for K=1 injects
            for c in range(NCLS):
                t_i, rr = divmod(c, 4)
                nc.sync.dma_start(gi_sc[t_i][32 * rr:32 * rr + 1, :],
                                  gi_sb[c:c + 1, :])

    # A-work static schedule over each 8-step window k (steps 8k..8k+7),
    # producing m' = k+2 (consumed at steps 8(k+2)..):
    #   r=0: issue gather(k+3); tanh+ones for m'   (gather was issued 8 steps
    #        ago, so it has had a full window to land)
    #   r=1: embT transposes + copies for m'
    #   r=2..7: two projection matmuls per step (12 total), DVE copy per block
    am = {"g": {2: g2}}

    def emit_awork(t):
        k, r = divmod(t, 8)
        m = k + 2
        if m >= 32:
            return
        if r == 0:
            if k + 3 < 32:
                am["g"][k + 3] = emit_gather(k + 3)
            am["th"] = emit_tanh_emb(m, am["g"].pop(m))
            am["psx"] = {}
        elif r == 1:
            am["emb"] = emit_embT(m, am["th"])
            xa = pXA.tile([128, G4], bf16, tag="xa", name="xa")
            xa_tiles[m] = xa
            am["xa"] = xa
        else:
            i = r - 2
            ea, eb = am["emb"]
            for f in (2 * i, 2 * i + 1):
                emit_xmm(m, ea, eb, am["xa"], am["psx"], f, copy_eng="dve")

    for t in range(S):
        m, r = divmod(t, 8)
        xa = xa_tiles[m]
        q, par = divmod(r, 2)

        # gate PSUM halves: A = hidden 0:256, B = hidden 256:512 per block
        psgA = pG.tile([128, 256], f32, tag="psgA", name="psgA")
        psgB = pG.tile([128, 256], f32, tag="psgB", name="psgB")
        selp = sel_sb[32 * q:32 * q + 32, 16 * par:16 * par + 16]

        # injects first: independent of H_prev, fill PE idle of prior chain
        for j in range(4):
            nc.tensor.matmul(
                psgA[32 * j:32 * j + 16, :], selp,
                xa[32 * q:32 * q + 32, 512 * j:512 * j + 256],
                start=True, stop=False, tile_position=(32 * q, 32 * j))
        for j in range(4):
            nc.tensor.matmul(
                psgB[32 * j:32 * j + 16, :], selp,
                xa[32 * q:32 * q + 32, 512 * j + 256:512 * j + 512],
                start=True, stop=False, tile_position=(32 * q, 32 * j))
        for kc in range(4):
            for j in range(4):
                nc.tensor.matmul(
                    psgA[32 * j:32 * j + 16, :],
                    H_prev[:, 16 * kc:16 * (kc + 1)],
                    whh_sb[kc][:, 512 * j:512 * j + 256],
                    start=False, stop=(kc == 3), tile_position=(0, 32 * j))
        for kc in range(4):
            for j in range(4):
                nc.tensor.matmul(
                    psgB[32 * j:32 * j + 16, :],
                    H_prev[:, 16 * kc:16 * (kc + 1)],
                    whh_sb[kc][:, 512 * j + 256:512 * j + 512],
                    start=False, stop=(kc == 3), tile_position=(0, 32 * j))

        T_t = pT.tile([128, 512], f32, tag="T", name="T")
        nc.scalar.activation(T_t[0:112, 0:256], psgA[0:112, :], Tanh)
        nc.scalar.activation(T_t[0:112, 256:512], psgB[0:112, :], Tanh)
        pstT = pTT.tile([128, 448], f32, tag="pstT", name="pstT")
        for k in range(4):
            tr(pstT[:, 112 * k:112 * (k + 1)], T_t[0:112, 128 * k:128 * (k + 1)], 112)
        Tv = pstT[:].rearrange("p (c w) -> p c w", w=112)
        ti, tf = Tv[:, :, 0:16], Tv[:, :, 32:48]
        tg, to = Tv[:, :, 64:80], Tv[:, :, 96:112]

        a_t = pW.tile([128, 64], f32, tag="a", name="a")
        v_t = pW.tile([128, 64], f32, tag="v", name="v")
        av = a_t[:].rearrange("p (c w) -> p c w", w=16)
        vv = v_t[:].rearrange("p (c w) -> p c w", w=16)
        zpv = z_prev[:].rearrange("p (c w) -> p c w", w=16)
        # tip must be separate: v=(ti+1)*tg can't be one STT because ti and
        # tg are both PSUM (single DVE PSUM read port).  It stays on DVE: an
        # ACT read of pstT's bank would serialize against the DVE reads
        # (bank-overlap tracking), putting it on the critical path.
        tip = pW.tile([128, 64], f32, tag="tip", name="tip")
        nc.vector.tensor_scalar_add(
            out=tip[:].rearrange("p (c w) -> p c w", w=16), in0=ti,
            scalar1=1.0)
        nc.vector.scalar_tensor_tensor(out=av, in0=tf, scalar=1.0, in1=zpv,
                                       op0=ADD, op1=MUL)
        nc.vector.tensor_tensor(
            out=vv, in0=tip[:].rearrange("p (c w) -> p c w", w=16),
            in1=tg, op=MUL)
        z_new = pZ.tile([128, 64], f32, tag="z", name="z")
        nc.vector.scalar_tensor_tensor(out=z_new[:], in0=a_t[:], scalar=0.5,
                                       in1=v_t[:], op0=MUL, op1=ADD)
        tc_t = pW.tile([128, 64], f32, tag="tc", name="tc")
        nc.scalar.activation(tc_t[:], z_new[:], Tanh, scale=0.5)
        H_new = pH.tile([128, 64], bf16, tag="H", name="H")
        nc.vector.scalar_tensor_tensor(
            out=H_new[:].rearrange("p (c w) -> p c w", w=16),
            in0=to, scalar=1.0,
            in1=tc_t[:].rearrange("p (c w) -> p c w", w=16),
            op0=ADD, op1=MUL)
        z_prev, H_prev = z_new, H_new

        emit_awork(t)
        if t >= 240:
            emit_dprep(t - 240)

    # decoder initial state (into pDP so scan pools can be released)
    nc.vector.tensor_scalar_mul(Hd0[:, 0:64], H_prev[:], 0.5)
    nc.vector.tensor_scalar_mul(Hd0[:, 64:128], bH[:], 0.5)

    # release scan-phase pools so the decoder gets PSUM banks back
    pW.release()
    pZ.release()
    pHt.release()
    pTT.release()
    pT.release()
    pG.release()
    pH.release()
    pPS.release()
    pXA.release()
    pA.release()

    # ======== Phase D: decoder (6 GRU steps + logits + log_softmax) ========
    pD = pool(name="pD", bufs=2)
    pDh = pool(name="pDh", bufs=2)
    pDg = pool(name="pDg", bufs=2, space="PSUM")   # psd0/psd1
    pDt = pool(name="pDt", bufs=1, space="PSUM")   # pstz/psn/psl

    dheat = pDt.tile([128, 512], f32, tag="dheat", name="dheat")

    def emit_dheat(n, pin=None):
        if pin is not None:
            nc.tensor.matmul(dheat[0:16, 0:128], ident[:, 0:16],
                             pin, start=True, stop=True)
            n -= 1
        for _ in range(n):
            nc.tensor.matmul(dheat[0:16, 0:128], identb[:, 0:16],
                             dwhh_sb[0][:, 0:128], start=True, stop=True)

    Hd = Hd0
    l_all = pD.tile([16, 12], f32, tag="lall", name="lall")

    for c in range(NCLS):
        t_i, rr = divmod(c, 4)
        # psd0: r,z blocks 0..3 on bands 0..3 (N=512); psd1: n blocks 4,5
        # split into N=256 halves on bands 0..3.
        psd0 = pDg.tile([128, 512], f32, tag="psd0", name="psd0")
        psd1 = pDg.tile([128, 256], f32, tag="psd1", name="psd1")
        # gi injects (K=1 from 32-aligned scattered rows)
        for ng in range(4):
            nc.tensor.matmul(
                psd0[32 * ng:32 * ng + 16, :],
                ones128[32 * rr:32 * rr + 1, :],
                gi_sc[t_i][32 * rr:32 * rr + 1, 512 * ng:512 * (ng + 1)],
                start=True, stop=False, tile_position=(32 * rr, 32 * ng))
        # NOTE: no gi inject into psd1 — gi_n enters OUTSIDE the r*h_n
        # product (n = tanh(i_n + r*h_n)); it is added later via giT.
        for kc in range(8):
            lh = Hd[:, 16 * kc:16 * (kc + 1)]
            for ng in range(4):
                nc.tensor.matmul(
                    psd0[32 * ng:32 * ng + 16, :], lh,
                    dwhh_sb[kc][:, 512 * ng:512 * (ng + 1)],
                    start=False, stop=False, tile_position=(0, 32 * ng))
        for ng in range(4):
            nc.tensor.matmul(
                psd0[32 * ng:32 * ng + 16, :], bias_stat[0:4, :],
                dwhh_sb[8][0:4, 512 * ng:512 * (ng + 1)],
                start=False, stop=True, tile_position=(0, 32 * ng))
        for kc in range(8):
            lh = Hd[:, 16 * kc:16 * (kc + 1)]
            for j in range(4):
                bb, hh = 4 + j // 2, j % 2
                nc.tensor.matmul(
                    psd1[32 * j:32 * j + 16, :], lh,
                    dwhh_sb[kc][:, 512 * bb + 256 * hh:512 * bb + 256 * hh + 256],
                    start=(kc == 0), stop=False, tile_position=(0, 32 * j))
        for j in range(4):
            bb, hh = 4 + j // 2, j % 2
            nc.tensor.matmul(
                psd1[32 * j:32 * j + 16, :], bias_stat[0:4, :],
                dwhh_sb[8][0:4, 512 * bb + 256 * hh:512 * bb + 256 * hh + 256],
                start=False, stop=True, tile_position=(0, 32 * j))

        # r,z: tanh then transpose to hidden-on-partitions
        Trz = pD.tile([128, 512], f32, tag="Trz", name="Trz")
        nc.scalar.activation(Trz[0:112, :], psd0[0:112, :], Tanh)
        pstz = pDt.tile([128, 448], f32, tag="pstz", name="pstz")
        for k in range(4):
            tr(pstz[:, 112 * k:112 * (k + 1)], Trz[0:112, 128 * k:128 * (k + 1)], 112)
        # n: scale by 0.5 (folds the later 0.5*sn) then transpose, and copy
        # the transposed result back to SBUF so sn's STT has only one PSUM
        # operand.
        hn_sb = pD.tile([128, 256], f32, tag="hn", name="hn")
        nc.vector.tensor_scalar_mul(hn_sb[0:112, :], psd1[0:112, :], 0.5)
        psn = pDt.tile([128, 224], f32, tag="psn", name="psn")
        for k in range(2):
            tr(psn[:, 112 * k:112 * (k + 1)], hn_sb[0:112, 128 * k:128 * (k + 1)], 112)
        nsb = pD.tile([128, 224], f32, tag="nsb", name="nsb")
        nc.vector.tensor_copy(nsb[:], psn[:])

        # layouts:
        #   r chunk k (k=4u+cc):   pstz free = 112*cc + 32*u          (+w)
        #   z chunk k (k=4u+cc):   pstz free = 112*cc + 64 + 32*u     (+w)
        #   n chunk k (k=4u+2v+w2): psn free = 112*w2 + 64*u + 32*v   (+w)
        #   sn/nT/d/e/hn2/Hd: natural order, free = 16*k (+w)
        zvT = pstz[:].rearrange("p (c w) -> p c w", w=112)
        nvT = nsb[:].rearrange("p (c w) -> p c w", w=112)
        sn = pD.tile([128, 128], f32, tag="sn", name="sn")
        for u in range(2):
            for v in range(2):
                # chunks 4u+2v+{0,1}: r at (c=2v+w2, woff 32u),
                # n at (tr=w2, woff 64u+32v); n already carries the 0.5
                base = 64 * u + 32 * v
                nc.vector.scalar_tensor_tensor(
                    out=sn[:, base:base + 32].rearrange(
                        "p (w2 w) -> p w2 w", w=16),
                    in0=zvT[:, 2 * v:2 * v + 2, 32 * u:32 * u + 16],
                    scalar=1.0,
                    in1=nvT[:, 0:2, base:base + 16],
                    op0=ADD, op1=MUL)
        # sn2 = sn + gi_n (broadcast over batch)
        sn2 = pD.tile([128, 128], f32, tag="sn2", name="sn2")
        gnv = giT[:].rearrange("p (g c) -> p g c", c=6)[:, :, c:c + 1]
        nc.vector.tensor_tensor(
            out=sn2[:].rearrange("p (g w) -> p g w", w=16),
            in0=sn[:].rearrange("p (g w) -> p g w", w=16),
            in1=gnv.to_broadcast([128, 8, 16]), op=ADD)
        nT = pD.tile([128, 128], f32, tag="nT", name="nT")
        nc.scalar.activation(nT[:], sn2[:], Tanh)
        d_t = pD.tile([128, 128], f32, tag="dt", name="dt")
        nc.vector.tensor_tensor(out=d_t[:], in0=Hd[:], in1=nT[:], op=SUB)
        # e = (tz+1)*d ; tz chunk k=4u+cc at pstz 112*cc + 64 + 32*u
        e_t = pD.tile([128, 128], f32, tag="et", name="et")
        for u in range(2):
            nc.vector.scalar_tensor_tensor(
                out=e_t[:, 64 * u:64 * u + 64].rearrange(
                    "p (c w) -> p c w", w=16),
                in0=zvT[:, :, 64 + 32 * u:80 + 32 * u], scalar=1.0,
                in1=d_t[:, 64 * u:64 * u + 64].rearrange(
                    "p (c w) -> p c w", w=16),
                op0=ADD, op1=MUL)
        hn2 = pD.tile([128, 128], f32, tag="hn2", name="hn2")
        nc.vector.scalar_tensor_tensor(out=hn2[:], in0=e_t[:], scalar=0.5,
                                       in1=nT[:], op0=MUL, op1=ADD)
        Hd_new = pDh.tile([128, 128], bf16, tag="Hd", name="Hd")
        nc.scalar.activation(Hd_new[:], hn2[:], Tanh)

        psl = pDt.tile([16, 2], f32, tag="psl", name="psl")
        for kc in range(8):
            nc.tensor.matmul(psl[:], Hd_new[:, 16 * kc:16 * (kc + 1)],
                             cls_sb[kc][:, 0:2], start=(kc == 0), stop=False)
        nc.tensor.matmul(psl[:], bias_stat[0:4, :], cls_sb[8][0:4, 0:2],
                         start=False, stop=True)
        nc.vector.tensor_copy(l_all[:, 2 * c:2 * c + 2], psl[:])
        Hd = Hd_new
        emit_dheat(12, pin=sn[:, 0:128])  # fill the late-chain window

    la = l_all[:].rearrange("p (c t) -> p c t", t=2)
    mx = pD.tile([16, 6], f32, tag="mx", name="mx")
    nc.vector.tensor_tensor(out=mx[:].rearrange("p (c o) -> p c o", o=1),
                            in0=la[:, :, 0:1], in1=la[:, :, 1:2], op=MAX)
    d0 = pD.tile([16, 12], f32, tag="d0", name="d0")
    d0v = d0[:].rearrange("p (c t) -> p c t", t=2)
    mxb = mx[:].rearrange("p (c o) -> p c o", o=1).to_broadcast([16, 6, 2])
    nc.vector.tensor_tensor(out=d0v, in0=la, in1=mxb, op=SUB)
    ex = pD.tile([16, 12], f32, tag="ex", name="ex")
    nc.scalar.activation(ex[:], d0[:], Exp)
    se = pD.tile([16, 6], f32, tag="se", name="se")
    nc.vector.tensor_reduce(out=se[:].rearrange("p (c o) -> p c o", o=1),
                            in_=ex[:].rearrange("p (c t) -> p c t", t=2),
                            op=ADD, axis=mybir.AxisListType.X)
    ls = pD.tile([16, 6], f32, tag="ls", name="ls")
    nc.scalar.activation(ls[:], se[:], Ln)
    ov = pD.tile([16, 12], f32, tag="ov", name="ov")
    lsb = ls[:].rearrange("p (c o) -> p c o", o=1).to_broadcast([16, 6, 2])
    nc.vector.tensor_tensor(out=ov[:].rearrange("p (c t) -> p c t", t=2),
                            in0=d0v, in1=lsb, op=SUB)
    nc.sync.dma_start(out_d[:].rearrange("c b t -> b c t"),
                      ov[:].rearrange("p (c t) -> p c t", t=2))

    pDt.release()
    pDg.release()
    pDh.release()
    pD.release()
    pDP.release()
    const.release()


def _prep_inputs(seq, classes, embed_W, embed_class_W, f_Wih, f_Whh, f_b,
                 b_Wih, b_Whh, b_b, d_Wih, d_Whh, d_bih, d_bhh, cls_W, cls_b):
    seq = np.asarray(seq)
    s4 = np.concatenate([np.full(H, 0.5), np.full(H, 0.5), np.ones(H),
                         np.full(H, 0.5)]).astype(np.float32)
    s3 = np.concatenate([np.full(H2, 0.5), np.full(H2, 0.5),
                         np.ones(H2)]).astype(np.float32)

    def padrows(a, rows):
        out = np.zeros((rows, a.shape[1]), np.float32)
        out[:a.shape[0]] = a
        return out

    wihT = padrows(np.concatenate(
        [(f_Wih * s4[:, None]).T, (f_b * s4)[None, :]], axis=0), 304)
    bwihT = padrows(np.concatenate(
        [(b_Wih * s4[:, None]).T, (b_b * s4)[None, :]], axis=0), 304)
    whhT = ((f_Whh * s4[:, None]) * 0.5).T.astype(np.float32)
    dwhhT = padrows(np.concatenate(
        [(d_Whh * s3[:, None]).T, (d_bhh * s3)[None, :]], axis=0), 1028)
    dwihT = padrows(np.concatenate(
        [(d_Wih * s3[:, None]).T, (d_bih * s3)[None, :]], axis=0), 516)
    clsT = padrows(np.concatenate(
        [np.asarray(cls_W, np.float32).T, np.asarray(cls_b, np.float32)[None, :]],
        axis=0), 1028)
    ecw = np.asarray(embed_class_W, np.float32)[np.asarray(classes)]

    sel = np.zeros((128, 32), np.float32)
    for blk in range(4):
        for i in range(16):
            sel[32 * blk + i, i] = 1.0
            sel[32 * blk + 16 + i, 16 + i] = 1.0

    shared = {
        "embedW": np.ascontiguousarray(np.asarray(embed_W, np.float32)),
        "wihT": _bf(wihT), "bwihT": _bf(bwihT), "whhT": _bf(whhT),
        "dwhhT": _bf(dwhhT), "dwihT": _bf(dwihT),
        "ecw": np.ascontiguousarray(ecw),
        "clsT": _bf(clsT),
        "sel": _bf(sel),
    }
    in_maps = []
    for c in range(NC):
        tok = np.asarray(seq[BL * c:BL * (c + 1), :], np.int32)  # [16, 256]
        idx = np.ascontiguousarray(
            tok.T.reshape(S * BL).reshape(32, 128).T.astype(np.int32))
        m = dict(shared)
        m["idx"] = idx
        in_maps.append(m)
    return in_maps


def kernel(**inputs):
    if "nc" not in _cache:
        _cache["nc"] = _build_program()
    nc = _cache["nc"]
    in_maps = _prep_inputs(**inputs)
    import os
    trace = bool(int(os.environ.get("BK_TRACE", "0")))
    res = run_bass_kernel_spmd(nc, in_maps, core_ids=list(range(NC)),
                               trace=trace)
    _cache["last_result"] = res
    outs = [res.results[c]["out"] for c in range(NC)]
    return np.concatenate(outs, axis=1).astype(np.float32)
